# revision 1
# baseline (speedup 1.0000x reference)
"""BitLinear (x @ ternary_kernel + bias) on 8 Trainium2 NeuronCores.

Strategy: data-parallel over the batch dim (8 batches -> 8 cores). Each core
computes out_b = x_b @ W for x_b [2048, 4096], W [4096, 4096], fp16 matmul
with fp32 PSUM accumulation (~2e-4 rel err vs fp32 reference; W is ternary
so it is exact in fp16).

Per-core kernel: x_b^T stays fully resident in SBUF (16 MiB as 16 m-tiles of
[128k x 32ko x 128m]); W streams as 8 column chunks of [128k x 32ko x 512u]
(4 MiB each, double-buffered), each reused across all 16 m-tiles so the PE
gets ~109us of dense matmuls per 11us prefetch and never goes cold. PSUM
tiles [128m x 512u] accumulate 32 matmuls over K, evicted via DVE copy and
DMA'd straight to the natural [2048, 4096] fp32 output layout.

Host-side prep (free wrt device time): fp16 cast + retile so every DMA is
fully contiguous in DRAM.
"""

import numpy as np

import concourse.bacc as bacc
import concourse.mybir as mybir
import concourse.tile as tile
from concourse.bass_utils import run_bass_kernel_spmd

B, T, D, U = 8, 2048, 4096, 4096
P = 128
KO = D // P      # 32 k-tiles of 128
MO = T // P      # 16 m-tiles of 128
NF = 512         # psum free dim (one bank)
NO = U // NF     # 8 n-chunks
N_CORES = 8

_F16 = np.float16

_cached_nc = None


def _build_program():
    nc = bacc.Bacc("TRN2", target_bir_lowering=False, debug=False,
                   num_devices=N_CORES)
    f16 = mybir.dt.float16
    f32 = mybir.dt.float32
    xt_d = nc.dram_tensor("xt", [MO, P, KO, P], f16,
                          kind="ExternalInput").ap()
    w_d = nc.dram_tensor("w", [NO, P, KO, NF], f16,
                         kind="ExternalInput").ap()
    out_d = nc.dram_tensor("out", [T, U], f32, kind="ExternalOutput").ap()

    with tile.TileContext(nc) as tc:
        KQ = KO // 4  # 8 k-tiles per W quarter-tile
        with (
            tc.tile_pool(name="xpool", bufs=MO) as xpool,
            tc.tile_pool(name="wpool", bufs=8) as wpool,
            tc.tile_pool(name="opool", bufs=4) as opool,
            tc.tile_pool(name="psum", bufs=8, space="PSUM") as psum_pool,
        ):
            # Emission order matters: only xt[0] + the first W quarter
            # (1 MiB) gate the first matmul; the other x tiles and W
            # quarters stream in behind and hide under compute.
            from concourse.tile_rust import add_dep_helper

            def load_w_chunk(no):
                qs, insts = [], []
                for q in range(4):
                    wq = wpool.tile([P, KQ, NF], f16, tag="w")
                    di = nc.sync.dma_start(
                        out=wq[:],
                        in_=w_d[no, :, q * KQ:(q + 1) * KQ, :])
                    qs.append(wq)
                    insts.append(di)
                return qs, insts

            xtiles = []
            xt = xpool.tile([P, KO, P], f16, tag="x")
            nc.sync.dma_start(out=xt[:], in_=xt_d[0])
            xtiles.append(xt)
            wt0, w0_insts = load_w_chunk(0)
            for mo in range(1, MO):
                xt = xpool.tile([P, KO, P], f16, tag="x")
                di = nc.sync.dma_start(out=xt[:], in_=xt_d[mo])
                # Keep these 15 loads out of the SDMA rings until the
                # gating first W quarter has landed, so it gets the HBM
                # bandwidth during the startup window.
                add_dep_helper(di.ins if hasattr(di, "ins") else di,
                               w0_insts[0].ins if hasattr(w0_insts[0], "ins")
                               else w0_insts[0],
                               reason="delay xt prefetch past first W quarter")
                xtiles.append(xt)
            for no in range(NO):
                wt = wt0 if no == 0 else load_w_chunk(no)[0]
                for mo in range(MO):
                    ps = psum_pool.tile([P, NF], f32)
                    for ko in range(KO):
                        nc.tensor.matmul(ps[:], lhsT=xtiles[mo][:, ko, :],
                                         rhs=wt[ko // KQ][:, ko % KQ, :],
                                         start=(ko == 0), stop=(ko == KO - 1))
                    ob = opool.tile([P, NF], f32)
                    nc.vector.tensor_copy(out=ob[:], in_=ps[:])
                    # scalar HWDGE queue: keeps output stores off the sync
                    # queue that feeds the critical x/W prefetches
                    nc.scalar.dma_start(
                        out=out_d[mo * P:(mo + 1) * P, no * NF:(no + 1) * NF],
                        in_=ob[:])
    nc.compile()
    return nc


def _get_program():
    global _cached_nc
    if _cached_nc is None:
        _cached_nc = _build_program()
    return _cached_nc


def make_in_maps(x, kernel):
    """Host-side shard + layout prep. Returns per-core input maps."""
    x = np.asarray(x)
    w = np.asarray(kernel)
    # w[no, p, ko, ni] = W[ko*128+p, no*512+ni]; shared by all cores.
    w_t = np.ascontiguousarray(
        w.astype(_F16).reshape(KO, P, NO, NF).transpose(2, 1, 0, 3))
    in_maps = []
    for b in range(B):
        # xt[mo, p, ko, mi] = x[b, mo*128+mi, ko*128+p]
        xb = np.ascontiguousarray(
            x[b].astype(_F16).reshape(MO, P, KO, P).transpose(0, 3, 2, 1))
        in_maps.append({"xt": xb, "w": w_t})
    return in_maps


def assemble_output(results, bias):
    bias = np.asarray(bias, dtype=np.float32)
    out = np.empty((B, T, U), dtype=np.float32)
    for b in range(B):
        out[b] = results[b]["out"]
    if np.any(bias):
        out += bias[None, None, :]
    return out


def kernel(x, kernel, bias):
    nc = _get_program()
    in_maps = make_in_maps(x, kernel)
    last_err = None
    for attempt in range(3):
        try:
            res = run_bass_kernel_spmd(nc, in_maps,
                                       core_ids=list(range(N_CORES)))
            return assemble_output(res.results, bias)
        except Exception as e:  # transient device wedge (NRT_EXEC_UNIT_...)
            last_err = e
            try:
                import jax
                jax.clear_caches()
                jax.extend.backend.clear_backends()
            except Exception:
                pass
    raise last_err



# revision 2
# speedup vs baseline: 1.5150x; 1.5150x over previous
"""BitLinear (x @ ternary_kernel + bias) on 8 Trainium2 NeuronCores.

Strategy: data-parallel over the batch dim (8 batches -> 8 cores). Each core
computes out_b = x_b @ W for x_b [2048, 4096], W [4096, 4096].

Mixed-precision contraction: the first K16 columns of K run as fp16 matmuls
(full accuracy), the last K8 columns run as fp8e4m3 DoubleRow matmuls at 2x
PE rate (W is ternary, exact in fp8; only x quantization adds noise).
K8 is chosen so max-rel-err stays ~1.8e-2 < 2e-2. PE cycles drop to
(KO16 + C8) / 32 of the all-fp16 kernel.

Per-core kernel: x stays fully resident in SBUF (fp16 tiles for the K16
range, fp8 [128, C8, 2, 128] tiles for the K8 range); W streams as 8 column
chunks (fp16 + fp8 parts, double-buffered), each reused across all 16
m-tiles. PSUM tiles [128m x 512u] accumulate C8 DoubleRow + KO16 fp16
matmuls, evicted via DVE copy and DMA'd to the natural [2048, 4096] fp32
output layout.

Host-side prep (free wrt device time): dtype casts + retile so every DMA is
fully contiguous in DRAM.
"""

import numpy as np
import ml_dtypes

import concourse.bacc as bacc
import concourse.mybir as mybir
import concourse.tile as tile
from concourse.bass_utils import run_bass_kernel_spmd

B, T, D, U = 8, 2048, 4096, 4096
P = 128
MO = T // P      # 16 m-tiles of 128
NF = 512         # psum free dim (one bank)
NO = U // NF     # 8 n-chunks
N_CORES = 8

K8 = 1792        # fp8 (DoubleRow) K-columns, at the END of the K range
C8 = K8 // 256   # fp8 chunks of 256 (two 128-subtiles per DoubleRow matmul)
K16 = D - K8
KO16 = K16 // P  # fp16 k-tiles of 128

_F16 = np.float16
_F8 = ml_dtypes.float8_e4m3
_DR = mybir.MatmulPerfMode.DoubleRow

_cached_nc = None


def _splits(n, parts):
    base = n // parts
    rem = n % parts
    sizes = [base + (1 if i < rem else 0) for i in range(parts)]
    bounds = [0]
    for s in sizes:
        bounds.append(bounds[-1] + s)
    return [(bounds[i], bounds[i + 1]) for i in range(parts) if sizes[i]]


def _build_program():
    nc = bacc.Bacc("TRN2", target_bir_lowering=False, debug=False,
                   num_devices=N_CORES)
    f8 = mybir.dt.float8e4
    f16 = mybir.dt.float16
    f32 = mybir.dt.float32
    xt16_d = nc.dram_tensor("xt16", [MO, P, KO16, P], f16,
                            kind="ExternalInput").ap()
    xt8_d = nc.dram_tensor("xt8", [MO, P, C8, 2, P], f8,
                           kind="ExternalInput").ap()
    w16_d = nc.dram_tensor("w16", [NO, P, KO16, NF], f16,
                           kind="ExternalInput").ap()
    w8_d = nc.dram_tensor("w8", [NO, P, C8, 2, NF], f8,
                          kind="ExternalInput").ap()
    out_d = nc.dram_tensor("out", [T, U], f32, kind="ExternalOutput").ap()

    q16 = _splits(KO16, 4)   # fp16 W chunk loaded as 4 k-quarters
    q8 = _splits(C8, 2)      # fp8 W chunk loaded as 2 halves

    with tile.TileContext(nc) as tc:
        with (
            tc.tile_pool(name="x16pool", bufs=MO) as x16pool,
            tc.tile_pool(name="x8pool", bufs=MO) as x8pool,
            tc.tile_pool(name="w16pool", bufs=2 * len(q16)) as w16pool,
            tc.tile_pool(name="w8pool", bufs=2 * len(q8)) as w8pool,
            tc.tile_pool(name="opool", bufs=4) as opool,
            tc.tile_pool(name="psum", bufs=8, space="PSUM") as psum_pool,
        ):
            from concourse.tile_rust import add_dep_helper

            def _ins(x):
                return x.ins if hasattr(x, "ins") else x

            def load_w16(no):
                tiles, insts = [], []
                for a, b in q16:
                    wq = w16pool.tile([P, b - a, NF], f16, tag="w16")
                    di = nc.sync.dma_start(out=wq[:], in_=w16_d[no, :, a:b, :])
                    tiles.append((a, wq))
                    insts.append(di)
                return tiles, insts

            def load_w8(no):
                tiles, insts = [], []
                for a, b in q8:
                    wq = w8pool.tile([P, b - a, 2, NF], f8, tag="w8")
                    di = nc.sync.dma_start(out=wq[:],
                                           in_=w8_d[no, :, a:b, :, :])
                    tiles.append((a, wq))
                    insts.append(di)
                return tiles, insts

            def w16_slice(tiles, ko):
                for a, wq in reversed(tiles):
                    if ko >= a:
                        return wq[:, ko - a, :]
                raise AssertionError

            def w8_slice(tiles, c):
                for a, wq in reversed(tiles):
                    if c >= a:
                        return wq[:, c - a, :, :]
                raise AssertionError

            # Startup order: the first DoubleRow matmul only needs xt8[0] +
            # the first w8 half; fp16 tiles follow. All other x tiles are
            # gated behind the first W loads so they don't starve the
            # critical path during the startup window.
            x8tiles, x16tiles = [], []
            xt8 = x8pool.tile([P, C8, 2, P], f8, tag="x8")
            nc.sync.dma_start(out=xt8[:], in_=xt8_d[0])
            x8tiles.append(xt8)
            wt8, w8_insts = load_w8(0)
            xt16 = x16pool.tile([P, KO16, P], f16, tag="x16")
            nc.sync.dma_start(out=xt16[:], in_=xt16_d[0])
            x16tiles.append(xt16)
            wt16, w16_insts = load_w16(0)
            gate = [w8_insts[0], w16_insts[0]]
            for mo in range(1, MO):
                xt8 = x8pool.tile([P, C8, 2, P], f8, tag="x8")
                d8 = nc.sync.dma_start(out=xt8[:], in_=xt8_d[mo])
                x8tiles.append(xt8)
                xt16 = x16pool.tile([P, KO16, P], f16, tag="x16")
                d16 = nc.sync.dma_start(out=xt16[:], in_=xt16_d[mo])
                x16tiles.append(xt16)
                for g in gate:
                    add_dep_helper(_ins(d8), _ins(g),
                                   reason="delay x prefetch past first W")
                    add_dep_helper(_ins(d16), _ins(g),
                                   reason="delay x prefetch past first W")

            for no in range(NO):
                if no > 0:
                    wt8, _ = load_w8(no)
                    wt16, _ = load_w16(no)
                for mo in range(MO):
                    ps = psum_pool.tile([P, NF], f32)
                    for c in range(C8):
                        nc.tensor.matmul(ps[:], lhsT=x8tiles[mo][:, c],
                                         rhs=w8_slice(wt8, c),
                                         start=(c == 0), stop=False,
                                         perf_mode=_DR)
                    for ko in range(KO16):
                        nc.tensor.matmul(ps[:], lhsT=x16tiles[mo][:, ko, :],
                                         rhs=w16_slice(wt16, ko),
                                         start=False, stop=(ko == KO16 - 1))
                    ob = opool.tile([P, NF], f32)
                    nc.vector.tensor_copy(out=ob[:], in_=ps[:])
                    # scalar HWDGE queue: keeps output stores off the sync
                    # queue that feeds the critical x/W prefetches
                    nc.scalar.dma_start(
                        out=out_d[mo * P:(mo + 1) * P, no * NF:(no + 1) * NF],
                        in_=ob[:])
    nc.compile()
    return nc


def _get_program():
    global _cached_nc
    if _cached_nc is None:
        _cached_nc = _build_program()
    return _cached_nc


def make_in_maps(x, kernel):
    """Host-side shard + layout prep. Returns per-core input maps."""
    x = np.asarray(x)
    w = np.asarray(kernel)
    # fp16 part: w16[no, p, ko, nf] = W[ko*128+p, no*512+nf]
    w16 = np.ascontiguousarray(
        w[:K16].astype(_F16).reshape(KO16, P, NO, NF).transpose(2, 1, 0, 3))
    # fp8 part: w8[no, p, c, i, nf] = W[K16 + c*256 + i*128 + p, no*512+nf]
    w8 = np.ascontiguousarray(
        w[K16:].astype(_F8).reshape(C8, 2, P, NO, NF).transpose(3, 2, 0, 1, 4))
    in_maps = []
    for b in range(B):
        xb = x[b]
        # xt16[mo, p, ko, mi] = x[b, mo*128+mi, ko*128+p]
        x16 = np.ascontiguousarray(
            xb[:, :K16].astype(_F16).reshape(MO, P, KO16, P)
            .transpose(0, 3, 2, 1))
        # xt8[mo, p, c, i, mi] = x[b, mo*128+mi, K16 + c*256 + i*128 + p]
        x8 = np.ascontiguousarray(
            xb[:, K16:].astype(_F8).reshape(MO, P, C8, 2, P)
            .transpose(0, 4, 2, 3, 1))
        in_maps.append({"xt16": x16, "xt8": x8, "w16": w16, "w8": w8})
    return in_maps


def assemble_output(results, bias):
    bias = np.asarray(bias, dtype=np.float32)
    out = np.empty((B, T, U), dtype=np.float32)
    for b in range(B):
        out[b] = results[b]["out"]
    if np.any(bias):
        out += bias[None, None, :]
    return out


def kernel(x, kernel, bias):
    nc = _get_program()
    in_maps = make_in_maps(x, kernel)
    last_err = None
    for attempt in range(3):
        try:
            res = run_bass_kernel_spmd(nc, in_maps,
                                       core_ids=list(range(N_CORES)))
            return assemble_output(res.results, bias)
        except Exception as e:  # transient device wedge (NRT_EXEC_UNIT_...)
            last_err = e
            try:
                import jax
                jax.clear_caches()
                jax.extend.backend.clear_backends()
            except Exception:
                pass
    raise last_err
